# revision 53
# baseline (speedup 1.0000x reference)
# Trainium2 Bass kernel for nn_BasicBlock (ShiftNet/AdderNet basic block).
#
# Reference computation (per full batch of 32 images):
#   y1 = conv3x3(x, quantize_pow2(w_shift1))          # power-of-two weights
#   z1 = -SAD3x3(y1, w_add1)                          # adder conv: -sum |patch - w|
#   a1 = relu(batchnorm_train(z1, g1, b1))            # batch stats over (N,H,W)
#   y2 = conv3x3(a1, quantize_pow2(w_shift2))
#   z2 = -SAD3x3(y2, w_add2)
#   out = relu(batchnorm_train(z2, g2, b2) + x)
#
# Strategy (8 NeuronCores, data-parallel over batch, 4 images/core):
#   - shift conv: 9 accumulating bf16 PE matmuls per output tile (pow2
#     weights are exact in bf16), shifted reads of a zero-padded bf16 plane,
#     evacuated directly into the next bf16 padded plane.
#   - adder conv via hat interpolation in weight space: |y - w| is piecewise
#     linear and convex in w, and w spans only ~[-0.15, 0.15].  With J nodes
#     v_j and hat (linear-interp) weights,  |y-w| ~= sum_j hat_j(w)*|y-v_j|.
#     Split around the v_mid=0 node (sum_j hat_j = 1):
#       sum_j hat_j*|y-v_j| = |y| + sum_{j!=mid} hat_j * delta_j(y),
#       delta_j(y) = |y-v_j| - |y| = s_j*(2*clamp(y, lo_j, hi_j) - (lo_j+hi_j))
#     The |y| "main" plane reduces through exact all-ones bf16 matmuls; the
#     clamp planes C_j = clamp(y, lo_j, hi_j) are bounded by the weight range
#     so fp8e4 holds them accurately, and the hat reduction runs as fp8
#     DoubleRow matmuls (2 nodes per pass, 0.5 cycles/row).  The per-channel
#     constant sum_j hat_j*s_j*(lo_j+hi_j) cancels under train-mode BN.
#     Interpolation is exact wherever |y| is outside the node range (the
#     common case: std(y)~1); the 0 node keeps the zero-pad halo terms |0-w|
#     exact.  Error ~5.6e-4 for J=9 quantile nodes (vs fp32 reference).
#   - batchnorm: per-core [mean, E[S^2]] via DVE bn_stats/bn_aggr + a 1KB
#     AllReduce across the 8 cores; scale/bias folded
#     (including the z = -S sign flip) into a single ScalarE
#     relu(scale*S + bias) with per-partition scale/bias.
import os
from contextlib import ExitStack

import numpy as np
import ml_dtypes

import concourse.bass as bass
import concourse.tile as tile
from concourse import bacc, mybir

F32 = mybir.dt.float32
BF16 = mybir.dt.bfloat16
FP8 = mybir.dt.float8e4
AF = mybir.ActivationFunctionType
ALU = mybir.AluOpType
DR = mybir.MatmulPerfMode.DoubleRow

# Problem constants (hardcoded per spec nn_BasicBlock_21131239097114)
N_FULL = 32
C_FULL = 128
H = W = 28
KK = 9           # 3x3 kernel positions
PH = PW = 30     # padded plane
PLANE = PH * PW  # 900
L = H * W        # 784
NTILE = 392      # matmul free dim = half an image plane (<=512 fp32 PSUM bank)
EPS = 1e-5
THRESH = 0.005
N_CORES = 8
N_IMG = N_FULL // N_CORES
# interpolation nodes per layer (incl. the 0 node).  Layer-1 errors are
# damped ~4x by layer-2's BN before reaching the output, so layer 1 can use
# a coarser grid.  fp8 DoubleRow processes node pairs: NPAIR_L = (J-1)//2.
J_L = (3, 5)
NPAIR_L = (1, 2)
NPAIR = max(NPAIR_L)

# perf attribution probes (outputs garbage): "noadd" = skip planes+hat
# matmuls, "nohat" = planes only, "noconv" = skip conv matmuls,
# "nobn" = skip the stats AllReduce (use per-core stats)
PROBE = os.environ.get("PROBE", "")


def shift_quant_np(w: np.ndarray) -> np.ndarray:
    """numpy mirror of reference.shift_quant (fp32 semantics)."""
    w = w.astype(np.float32)
    aw = np.abs(w)
    q = np.sign(w) * np.exp2(np.round(np.log2(np.maximum(aw, np.float32(1e-10)))))
    q = np.where(aw < np.float32(THRESH), np.float32(0.0), q).astype(np.float32)
    return q


def make_nodes(w: np.ndarray, Jl: int):
    """Per-layer node grid: weight quantiles, closest-to-0 node snapped to 0."""
    wv = np.asarray(w, np.float32).ravel()
    lo, hi = float(wv.min()), float(wv.max())
    span = hi - lo
    nodes = np.quantile(wv, np.linspace(0, 1, Jl))
    nodes[0] = lo - 0.001 * span
    nodes[-1] = hi + 0.001 * span
    j0 = int(np.argmin(np.abs(nodes)))
    nodes[j0] = 0.0
    nodes = nodes.astype(np.float32)
    assert len(np.unique(nodes)) == Jl
    return nodes, j0


def hat_matrices(w_kk: np.ndarray, nodes: np.ndarray) -> np.ndarray:
    """w_kk: [ci, co] -> H: [J, ci, co] linear-interp hat weights."""
    Jn = len(nodes)
    wc = np.clip(w_kk, nodes[0], nodes[-1])
    idx = np.clip(np.searchsorted(nodes, wc, side="right") - 1, 0, Jn - 2)
    lam = (wc - nodes[idx]) / (nodes[idx + 1] - nodes[idx])
    Hm = np.zeros((Jn, *w_kk.shape), np.float32)
    for j in range(Jn):
        Hm[j] += np.where(idx == j, 1.0 - lam, 0.0)
        Hm[j] += np.where(idx + 1 == j, lam, 0.0)
    return Hm


def build_body(tc, out_ap, x_ap, wq_ap, hat_ap, gb_ap, nodes,
               c: int, n_img: int, n_cores: int, repeat: int = 1):
    nc = tc.nc
    PL = n_img * PLANE
    n_t = 2 * n_img                    # psum tiles per adder phase
    inv_cores = 1.0 / float(n_cores)   # stats are AllReduced per-core means
    # per-layer clamp bounds for the delta planes (0 node excluded)
    clamps = []
    for layer in range(2):
        nd, jmid = nodes[layer]
        js = [j for j in range(J_L[layer]) if j != jmid]
        clamps.append([(min(float(nd[j]), 0.0), max(float(nd[j]), 0.0))
                       for j in js])

    with ExitStack() as ctx:
        sing = ctx.enter_context(tc.tile_pool(name="sing", bufs=1))
        dram = ctx.enter_context(tc.tile_pool(name="drampool", bufs=1, space="DRAM"))

        x_pad = sing.tile([c, PL + 64], F32, tag="x_pad")
        x16 = sing.tile([c, PL], BF16, tag="x16")           # conv1 rhs
        y16 = sing.tile([c, PL], BF16, tag="y16")           # conv out (y1/y2)
        a16 = sing.tile([c, PL], BF16, tag="a16")           # conv2 rhs
        m16s = sing.tile([c, PL + 2], BF16, tag="m16s")     # |y| at offset 1
        m16c = sing.tile([c, PL], BF16, tag="m16c")         # |y| at offset 0
        brow = sing.tile([c, PL], BF16, tag="brow")         # 3-tap row sum
        b16 = sing.tile([c, PL], BF16, tag="b16")           # 3x3 box sum of |y|
        c8 = sing.tile([c, 2 * NPAIR, PL], FP8, tag="c8")   # clamp planes
        S_sb = sing.tile([c, n_img, L], F32, tag="S_sb")    # reused: S1 then S2
        o_sb = sing.tile([c, n_img, L], F32, tag="o_sb")
        wq_sb = sing.tile([c, 2, KK, c], BF16, tag="wq_sb")
        hat_sb = sing.tile([c, 2, KK, NPAIR, 2, c], FP8, tag="hat_sb")
        allones = sing.tile([c, c], BF16, tag="allones")
        gb_sb = sing.tile([c, 4], F32, tag="gb_sb")
        consts = sing.tile([c, 3], F32, tag="consts")       # [0, eps, 1]
        stats6 = sing.tile([c, n_t, 6], F32, tag="stats6")  # bn_stats groups
        mv_sb = sing.tile([c, 2], F32, tag="mv_sb")         # [mean, var]
        stats = sing.tile([c, 2], F32, tag="stats")
        statsg = sing.tile([c, 2], F32, tag="statsg")
        bnw = sing.tile([c, 12], F32, tag="bnw")

        for t in (x_pad, x16, y16, a16, m16s, brow):
            nc.vector.memset(t[:, :], 0.0)
        nc.vector.memset(consts[:, 0:1], 0.0)
        nc.vector.memset(consts[:, 1:2], float(EPS))
        nc.vector.memset(consts[:, 2:3], 1.0)
        nc.vector.memset(allones[:, :], 1.0)
        zero_c, eps_c, ones_c = consts[:, 0:1], consts[:, 1:2], consts[:, 2:3]

        def pview(t):
            return t[:, :PL].rearrange("p (n ph pw) -> p n ph pw", ph=PH, pw=PW)

        xv = pview(x_pad)
        for n in range(n_img):
            nc.sync.dma_start(out=xv[:, n, 1:1 + H, 1:1 + W],
                              in_=x_ap[n].rearrange("c h w -> c h w"))
        nc.vector.tensor_copy(x16[:, :], x_pad[:, 0:PL])   # bf16 conv1 rhs
        nc.sync.dma_start(out=wq_sb[:, :, :, :],
                          in_=wq_ap.rearrange("l k i o -> i l k o"))
        nc.sync.dma_start(out=hat_sb[:, :, :, :, :, :], in_=hat_ap)
        nc.sync.dma_start(out=gb_sb[:, :], in_=gb_ap)

        HLF = (n_img // 2) * PLANE     # flat plane elems per image-half

        def layer_pass(layer: int, src16):
            """Full layer (shift conv + adder + stats), software-pipelined in
            two image halves so ACT/DVE producer work overlaps PE matmuls.
            S_sb[co,n,l] ~= sum_{ci,kk} |y - w| (minus a per-co constant that
            BN cancels) via |y|-box-sum + hat-weighted clamp-delta planes."""
            srcv = pview(src16)
            dstv = pview(y16)
            bv = pview(b16)
            cv = c8[:, :, :].rearrange("p q (n ph pw) -> p q n ph pw",
                                       ph=PH, pw=PW)
            with tc.tile_pool(name=f"psc{layer}", bufs=2, space="PSUM") as pp, \
                 tc.tile_pool(name=f"psa{layer}", bufs=6, space="PSUM") as pa:
                for g in range(2):
                    gH = g * HLF
                    # ---- shift conv for this half (tile-by-tile so each
                    # PSUM tile evacuates while the next streams) ----
                    for t in range(4 * g, 4 * g + 4):
                        n, hf = divmod(t, 2)
                        h0 = hf * 14
                        ps = pp.tile([c, NTILE], F32, tag="cps")
                        if PROBE == "noconv":
                            nc.vector.memset(ps[:, :], 0.0)
                        else:
                            for kk in range(KK):
                                dh, dw = divmod(kk, 3)
                                rhs = srcv[:, n, h0 + dh:h0 + dh + 14,
                                           dw:dw + W]
                                nc.tensor.matmul(
                                    ps[:, :], lhsT=wq_sb[:, layer, kk, :],
                                    rhs=rhs,
                                    start=(kk == 0), stop=(kk == KK - 1))
                        nc.scalar.activation(
                            out=dstv[:, n, 1 + h0:15 + h0, 1:1 + W],
                            in_=ps[:, :].rearrange("p (a b) -> p a b", a=14),
                            func=AF.Copy)
                    # ---- producers for this half (overlap next half's conv) --
                    if PROBE != "noadd":
                        # clamp planes first: they are single DVE ops and gate
                        # the DR matmuls, which open each PSUM group
                        for q, (lo, hi) in enumerate(clamps[layer]):
                            nc.vector.tensor_scalar(
                                out=c8[:, q, gH:gH + HLF],
                                in0=y16[:, gH:gH + HLF],
                                scalar1=lo, scalar2=hi,
                                op0=ALU.max, op1=ALU.min)
                        # y16 halo is 0 -> |y| halo 0 and clamp(0)=0: the halo
                        # contribution is the delta constant, which BN cancels.
                        # Two |y| copies (offsets 0/1) keep every box-sum DVE
                        # read 4B-aligned (2x packing mode).
                        nc.scalar.activation(out=m16s[:, gH + 1:gH + HLF + 1],
                                             in_=y16[:, gH:gH + HLF],
                                             func=AF.Abs, bias=zero_c,
                                             scale=1.0)
                        nc.scalar.activation(out=m16c[:, gH:gH + HLF],
                                             in_=y16[:, gH:gH + HLF],
                                             func=AF.Abs, bias=zero_c,
                                             scale=1.0)
                        # row pass: brow[i] = |y|[i-1] + |y|[i] + |y|[i+1]
                        # (last 2 elems stay 0 from init = correct halo value)
                        nc.vector.tensor_add(brow[:, gH:gH + HLF - 2],
                                             m16s[:, gH:gH + HLF - 2],
                                             m16s[:, gH + 2:gH + HLF])
                        nc.vector.tensor_add(brow[:, gH:gH + HLF - 2],
                                             brow[:, gH:gH + HLF - 2],
                                             m16c[:, gH:gH + HLF - 2])
                        # col pass: b16[i] = brow[i-30] + brow[i] + brow[i+30]
                        nc.vector.tensor_add(b16[:, gH + 30:gH + HLF - 30],
                                             brow[:, gH:gH + HLF - 60],
                                             brow[:, gH + 60:gH + HLF])
                        nc.vector.tensor_add(b16[:, gH + 30:gH + HLF - 30],
                                             b16[:, gH + 30:gH + HLF - 30],
                                             brow[:, gH + 30:gH + HLF - 30])
                # ---- adder matmuls + S evacuation, half by half ----
                for g in range(2):
                    Ts = {}
                    for t in range(4 * g, 4 * g + 4):
                        Ts[t] = pa.tile([c, 512], F32, tag="aps",
                                        name=f"aps{layer}_{t}")
                    if PROBE in ("noadd", "nohat"):
                        for t in range(4 * g, 4 * g + 4):
                            nc.vector.memset(Ts[t][:, :], 0.0)
                    else:
                        # delta first: fp8 DoubleRow over node pairs (gated
                        # only on the cheap clamp planes, so PE starts early)
                        npair = NPAIR_L[layer]
                        for p in range(npair):
                            for kk in range(KK):
                                dh, dw = divmod(kk, 3)
                                lhsT = hat_sb[:, layer, kk, p, :, :]
                                for t in range(4 * g, 4 * g + 4):
                                    n, hf = divmod(t, 2)
                                    h0 = hf * 14
                                    rhs = cv[:, 2 * p:2 * p + 2, n,
                                             h0 + dh:h0 + dh + 14, dw:dw + W]
                                    nc.tensor.matmul(Ts[t][:, 0:NTILE],
                                                     lhsT=lhsT, rhs=rhs,
                                                     perf_mode=DR,
                                                     start=(p == 0
                                                            and kk == 0),
                                                     stop=False)
                        # main: one all-ones matmul of the 3x3 box sum of |y|
                        # closes each accumulation group
                        for t in range(4 * g, 4 * g + 4):
                            n, hf = divmod(t, 2)
                            h0 = hf * 14
                            rhs = bv[:, n, 1 + h0:15 + h0, 1:1 + W]
                            nc.tensor.matmul(Ts[t][:, 0:NTILE],
                                             lhsT=allones[:, :], rhs=rhs,
                                             start=False, stop=True)
                    # evacuate PSUM -> SBUF; per-tile batch stats on DVE
                    # (overlap the next half's matmuls)
                    for t in range(4 * g, 4 * g + 4):
                        n, hf = divmod(t, 2)
                        h0 = hf * 14
                        sv = S_sb[:, n, h0 * W:(h0 + 14) * W]
                        nc.scalar.activation(out=sv, in_=Ts[t][:, 0:NTILE],
                                             func=AF.Copy)
                        nc.vector.bn_stats(out=stats6[:, t, :], in_=sv)
            # aggregate to per-core [mean, E[S^2]] for the AllReduce
            nc.vector.bn_aggr(out=mv_sb[:, :], in_=stats6[:, :, :])
            nc.vector.tensor_copy(stats[:, 0:1], mv_sb[:, 0:1])
            nc.vector.tensor_mul(stats[:, 1:2], mv_sb[:, 0:1], mv_sb[:, 0:1])
            nc.vector.tensor_add(stats[:, 1:2], stats[:, 1:2], mv_sb[:, 1:2])

        def bn_scales(layer: int):
            """AllReduce stats; return ([c,1] scale, [c,1] bias) APs such that
            bn_out = scale*S + bias  (includes the z = -S sign fold)."""
            cin = dram.tile([c, 2], F32, tag=f"cin{layer}")
            nc.gpsimd.dma_start(out=cin[:, :], in_=stats[:, :])
            if n_cores > 1 and PROBE != "nobn":
                cout = dram.tile([c, 2], F32, tag=f"cout{layer}")
                nc.gpsimd.collective_compute(
                    "AllReduce", ALU.add,
                    replica_groups=[list(range(n_cores))],
                    ins=[cin.opt()], outs=[cout.opt()])
                nc.gpsimd.dma_start(out=statsg[:, :], in_=cout[:, :])
            else:
                nc.gpsimd.dma_start(out=statsg[:, :], in_=cin[:, :])

            def col(i):
                return bnw[:, i:i + 1]
            v = nc.vector
            v.tensor_scalar_mul(col(0), statsg[:, 0:1], inv_cores)      # mean(S)
            v.tensor_scalar_mul(col(1), statsg[:, 1:2], inv_cores)      # E[S^2]
            v.tensor_mul(col(2), col(0), col(0))                        # mean^2
            v.tensor_sub(col(3), col(1), col(2))                        # var
            nc.scalar.activation(out=col(4), in_=col(3), func=AF.Sqrt,
                                 bias=eps_c)                            # sqrt(var+eps)
            v.reciprocal(col(5), col(4))                                # r0 ~ rsqrt
            v.tensor_scalar_add(col(6), col(3), float(EPS))             # v = var+eps
            v.tensor_mul(col(7), col(5), col(5))                        # r0^2
            v.tensor_mul(col(7), col(7), col(6))                        # v*r0^2
            v.tensor_scalar(out=col(7), in0=col(7), scalar1=-0.5, scalar2=1.5,
                            op0=ALU.mult, op1=ALU.add)                  # 1.5-0.5*v*r0^2
            v.tensor_mul(col(5), col(5), col(7))                        # refined rsqrt
            g = gb_sb[:, 2 * layer:2 * layer + 1]
            b = gb_sb[:, 2 * layer + 1:2 * layer + 2]
            v.tensor_mul(col(8), g, col(5))                             # gamma*r
            v.tensor_scalar_mul(col(9), col(8), -1.0)                   # scale=-gamma*r
            v.tensor_mul(col(10), col(0), col(8))                       # mu*gamma*r
            v.tensor_add(col(10), col(10), b)                           # bias
            return col(9), col(10)

        out_v = out_ap.rearrange("n c h w -> c n (h w)")
        for _rep in range(repeat):
            # ---- layer 1 ----
            layer_pass(0, x16)
            scale1, bias1 = bn_scales(0)
            av = pview(a16)[:, :, 1:1 + H, 1:1 + W]
            sve = S_sb[:, :, :].rearrange("p n (h w) -> p n h w", h=H)
            nc.scalar.activation(out=av, in_=sve, func=AF.Relu,
                                 scale=scale1, bias=bias1)

            # ---- layer 2 ----
            layer_pass(1, a16)
            scale2, bias2 = bn_scales(1)

            # out = relu(scale2*S2 + bias2 + x), per image so DMA overlaps
            ov = o_sb[:, :, :].rearrange("p n (h w) -> p n h w", h=H)
            for n in range(n_img):
                nc.vector.tensor_scalar(out=o_sb[:, n, :],
                                        in0=S_sb[:, n, :],
                                        scalar1=scale2, scalar2=bias2,
                                        op0=ALU.mult, op1=ALU.add)
                nc.vector.tensor_add(ov[:, n], ov[:, n],
                                     xv[:, n, 1:1 + H, 1:1 + W])
                nc.scalar.activation(out=o_sb[:, n, :], in_=o_sb[:, n, :],
                                     func=AF.Relu, bias=zero_c)
                nc.sync.dma_start(out=out_v[:, n, :], in_=o_sb[:, n, :])


def prep_weights(w_shift1, w_add1, w_shift2, w_add2, bn1_gamma, bn1_beta,
                 bn2_gamma, bn2_beta, c: int):
    """Host-side packing. Returns dict of device input arrays (minus x)."""
    wq = np.zeros((2, KK, c, c), ml_dtypes.bfloat16)         # pow2: exact bf16
    for layer, w in ((0, w_shift1), (1, w_shift2)):
        q = shift_quant_np(np.asarray(w, np.float32))       # [co, ci, kh, kw]
        for kk in range(KK):
            kh, kw = divmod(kk, 3)
            wq[layer, kk] = q[:, :, kh, kw].T                # [ci, co]
    # fp8 DoubleRow hats: [ci, layer, kk, pair, 2, co] = 2*s_j*hat_j
    hats = np.zeros((c, 2, KK, NPAIR, 2, c), ml_dtypes.float8_e4m3fn)
    nodes = []
    for layer, w in ((0, w_add1), (1, w_add2)):
        w = np.asarray(w, np.float32)                        # [co, ci, kh, kw]
        nd, jmid = make_nodes(w, J_L[layer])
        nodes.append((nd, jmid))
        js = [j for j in range(J_L[layer]) if j != jmid]
        for kk in range(KK):
            kh, kw = divmod(kk, 3)
            Hm = hat_matrices(w[:, :, kh, kw].T, nd)         # [J, ci, co]
            for qi, j in enumerate(js):
                sj = 1.0 if nd[j] <= 0.0 else -1.0           # sign(v_mid - v_j)
                p, q = divmod(qi, 2)
                hats[:, layer, kk, p, q, :] = (2.0 * sj * Hm[j]).astype(
                    ml_dtypes.float8_e4m3fn)
    gb = np.stack([np.asarray(v, np.float32) for v in
                   (bn1_gamma, bn1_beta, bn2_gamma, bn2_beta)], axis=1)
    return {"wq": np.ascontiguousarray(wq),
            "hats": np.ascontiguousarray(hats),
            "gb": np.ascontiguousarray(gb),
            "_nodes": nodes}


def build_program(c: int, n_img: int, n_cores: int, nodes, repeat: int = 1):
    nc = bacc.Bacc("TRN2", target_bir_lowering=False, debug=False,
                   num_devices=n_cores)
    x_t = nc.dram_tensor("x", [n_img, c, H, W], F32, kind="ExternalInput")
    wq_t = nc.dram_tensor("wq", [2, KK, c, c], BF16, kind="ExternalInput")
    hat_t = nc.dram_tensor("hats", [c, 2, KK, NPAIR, 2, c], FP8,
                           kind="ExternalInput")
    gb_t = nc.dram_tensor("gb", [c, 4], F32, kind="ExternalInput")
    out_t = nc.dram_tensor("out", [n_img, c, H, W], F32, kind="ExternalOutput")
    with tile.TileContext(nc) as tc:
        build_body(tc, out_t.ap(), x_t.ap(), wq_t.ap(), hat_t.ap(),
                   gb_t.ap(), nodes, c, n_img, n_cores, repeat=repeat)
    nc.compile()
    return nc


def run(inputs: dict, trace: bool = False):
    from concourse.bass_utils import run_bass_kernel_spmd
    x = np.ascontiguousarray(np.asarray(inputs["x"], np.float32))
    n, c = x.shape[0], x.shape[1]
    n_img = n // N_CORES
    host = prep_weights(inputs["w_shift1"], inputs["w_add1"],
                        inputs["w_shift2"], inputs["w_add2"],
                        inputs["bn1_gamma"], inputs["bn1_beta"],
                        inputs["bn2_gamma"], inputs["bn2_beta"], c)
    nodes = host.pop("_nodes")
    nc = build_program(c, n_img, N_CORES, nodes)
    in_maps = []
    for k in range(N_CORES):
        m = dict(host)
        m["x"] = np.ascontiguousarray(x[k * n_img:(k + 1) * n_img])
        in_maps.append(m)
    res = run_bass_kernel_spmd(nc, in_maps, core_ids=list(range(N_CORES)),
                               trace=trace)
    out = np.concatenate([r["out"] for r in res.results], axis=0)
    return out, res


def kernel(**inputs) -> np.ndarray:
    return run(inputs)[0]
